# revision 5
# baseline (speedup 1.0000x reference)
"""Trainium2 Bass kernel v3 for nn_MCPBRNN_SW_Variant_Routing_Norm.

Windowed-Picard solver for the scalar recurrence c' = f(c)c + u with
f(c) = 1 - oo1*sigmoid(w c + b0), outputs snapshotted at each row end.

v3 structure (all compute on the Vector engine, ~10 critical-path
instructions):
- quadratic model P(c) ~= f(c) fitted at build time over the reachable
  range [0, cmax]; Picard iterates scan (HW tensor_tensor_scan) +
  ONE fused custom-DVE op per f-update:
      POLY2C: out = (Src0*C0 + C1)*Src0 + C2   (= P(c) via Horner)
  registered into dve_ops.OPS at import (v3-lowered, sha pinned at
  registration time; CoreSim-validated and HW-tested).
- Picard seeded at f(cbar), cbar = fixed point under the mean input --
  worth one full iteration vs the f(0) seed: K=4 reaches ~6e-4 max rel
  err (gate 2e-2).
- the window is extended by 3 columns so the final scan itself emits
  [C, oo*C, oo, fC]: col L multiplies by oo (copied in), col L+1 adds
  oo (poly output written there directly), col L+2 is the STATIC
  affine s -> 1-s (f=-1, u=1).  Zero post-scan compute; the output DMA
  reads c[:, L-1:L+3].
- input and output DMAs both on SP/HWDGE (cheapest fixed-latency DGE
  path); the single cross-engine wait (input DMA -> DVE) rides a plain
  tensor_copy junction because TensorScalarPtr-encoded DVE ops cannot
  carry sync waits.
"""

import numpy as np

_CACHE = {}
_POLY2C = None


def _get_poly2c():
    """Register the fused Horner-quadratic custom-DVE op (idempotent)."""
    global _POLY2C
    if _POLY2C is not None:
        return _POLY2C
    import concourse.dve_ops as dvo
    from concourse.dve_spec import C0, C1, C2, Spec, Src0, lower
    from concourse.dve_table_gen import dve_ver_for
    from concourse.dve_uop import DveOpSpec

    name = "POLY2C_ANT"
    existing = [op for op in dvo.OPS if op.name == name]
    if existing:
        _POLY2C = existing[0]
        return _POLY2C

    spec = Spec(
        body=(Src0 * C0 + C1) * Src0 + C2,
        reference=lambda in0, in1, c0, c1, c2: (
            (in0.astype(np.float32) * np.float32(c0) + np.float32(c1))
            * in0.astype(np.float32)
            + np.float32(c2)
        ).astype(np.float32),
    )
    row = max(dvo._SUB_OPCODE_FOR_NAME.values()) + 1
    assert row < 0x20
    dvo._SUB_OPCODE_FOR_NAME[name] = row
    shas = {}
    for ver in ("v3", "v4"):
        s = DveOpSpec(name=name, opcode=row, uops=lower(spec, ver=ver), rd1_en=False)
        shas[ver] = s.sha(ver)
    op = dvo.DveOp(name, spec, subdim=False, uops_sha=shas)
    dvo.OPS.append(op)
    dvo.CUSTOM_DVE_SPECS[name] = spec  # CoreSim reference-fn registry
    _POLY2C = op
    return op


def _build(B, T, time_lag, L, K, p0, p1, p2, fseed):
    import concourse.bacc as bacc
    import concourse.mybir as mybir
    from concourse.tile import TileContext

    poly2c = _get_poly2c()

    f32 = mybir.dt.float32
    i16 = mybir.dt.int16
    R = B - time_lag
    W = L + 3
    Wc = L - 1 + 64  # c-tile width: scan cols 0..W-1 + scatter pad to 64
    mult = mybir.AluOpType.mult
    add = mybir.AluOpType.add
    q0, q1, q2 = 1.0 - p0, -p1, -p2  # oo(c) = 1 - P(c)

    nc = bacc.Bacc()
    x = nc.dram_tensor("x", [B, T], f32, kind="ExternalInput")
    # 64-elem rows: dma_scatter_add needs elem_size*dtype % 256B == 0.
    # Host reads rows 0..R-1, cols 0..3 = [C, oo*C, oo, fC]; slots
    # 62/63 land in scratch rows 62/63 (zeros from zeroed partitions).
    out = nc.dram_tensor("out", [64, 64], f32, kind="ExternalOutput")

    with TileContext(nc) as tc:
        with tc.tile_pool(name="pool", bufs=1) as pool:
            u = pool.tile([R, W], f32)
            f = pool.tile([R, W], f32)
            # 128 partitions (scatter slot space), 3D so the scatter's
            # [128, 1, 64] src view is expressible
            c3 = pool.tile([128, 1, Wc], f32)
            idx = pool.tile([128, 4], i16)
            dscr = pool.tile([R, 1], f32)

            # window for output i: u indices T-1-L .. T-2 of row time_lag+i
            nc.sync.dma_start(out=u[:, 0:L], in_=x[time_lag:B, T - 1 - L : T - 1])

            # zero the scatter-add destination (off the critical path: both
            # DMAs fit well inside the input window + compute span)
            z = pool.tile([64, 64], f32)
            nc.vector.memset(z[:, :], 0.0)
            nc.sync.dma_start(out=out[:, :], in_=z[:, :])

            # Output path: SWDGE descriptors prepped during the input-DMA
            # window; trigger fires after the last scan (deferred src dep).
            # idx slot j (partition j%16, col j//16) -> dram row j; the
            # output buffer is a donated zero buffer, so += is a write.
            nc.gpsimd.memset(idx[:, :], 0)
            nc.gpsimd.iota(idx[0:16, :], pattern=[[16, 4]], base=0, channel_multiplier=1)
            dma_sem = nc.alloc_semaphore("scatter_dma")
            nc.gpsimd.dma_scatter_add(
                out[:, :], c3[:, 0:1, L - 1 : L - 1 + 64], idx[:, :],
                64, 64, 64,
                prepare_only=True, sem=dma_sem,
            )

            # init (hidden under the input-DMA latency):
            #   u: [..u.., 0, oo*, 1]   f: [fseed.., 0|oo*, 0|Cprev*, -1]
            # (* = written before the last scan; 0-defined until then)
            nc.vector.memset(c3[:, 0, 0:Wc], 0.0)
            nc.vector.memset(u[:, L : L + 2], 0.0)
            nc.vector.memset(u[:, L + 2 : W], 1.0)
            nc.vector.memset(f[:, 0:L], fseed)
            nc.vector.memset(f[:, L : L + 2], 0.0)
            nc.vector.memset(f[:, L + 2 : W], -1.0)
            # plain-op junction absorbs the input-DMA wait
            nc.vector.tensor_copy(dscr[:, :], u[:, 0:1])

            c = c3[0:R, 0, 0:W]
            for k in range(K):
                if k == K - 1:
                    # craft the 3 extra columns from the prev iterate's C:
                    #   col L: f=oo, u=0 -> oo*C; col L+1: f=0, u=oo -> oo;
                    #   col L+2: f=-1, u=1 -> 1-oo = fC
                    Cp = c[:, L - 1 : L]
                    nc.vector._custom_dve(
                        poly2c, out=u[:, L + 1 : L + 2], in0=Cp,
                        s0=q2, s1=q1, imm2=q0,
                    )
                    nc.vector.tensor_copy(f[:, L : L + 1], u[:, L + 1 : L + 2])
                # c_t = f_t * c_{t-1} + u_t along the free dim, c_{-1} = 0
                nc.vector.tensor_tensor_scan(
                    out=c[:, :], data0=f[:, :], data1=u[:, :],
                    initial=0.0, op0=mult, op1=add,
                )
                if k < K - 1:
                    # f_t = P(c_{t-1}) for t in 1..L-1, one fused op
                    nc.vector._custom_dve(
                        poly2c, out=f[:, 1:L], in0=c[:, 0 : L - 1],
                        s0=p2, s1=p1, imm2=p0,
                    )

            nc.gpsimd.trigger_dma(count=None)

    nc.finalize()
    return nc


def _params(inputs):
    x = np.ascontiguousarray(np.asarray(inputs["x"], dtype=np.float32))
    time_lag = int(inputs["time_lag"])
    p_norm = float(np.asarray(inputs["p_norm"]).reshape(-1)[0])
    w_r_yom = float(np.asarray(inputs["w_r_yom"]).reshape(-1)[0])
    w_r_yfm = float(np.asarray(inputs["w_r_yfm"]).reshape(-1)[0])
    b0 = float(np.asarray(inputs["b0_yom"]).reshape(-1)[0])
    w_b1 = float(np.asarray(inputs["w_b1_yom"]).reshape(-1)[0])

    oo1 = float(np.exp(w_r_yom) / (np.exp(w_r_yom) + np.exp(w_r_yfm)))
    w = w_b1 / p_norm
    return x, time_lag, oo1, w, b0


def _fit(oo1, w, b0, umax, umean):
    """Window length, quadratic coefficients, Picard seed value."""
    sig = lambda v: 1.0 / (1.0 + np.exp(-v))
    f0 = 1.0 - oo1 * sig(b0)
    # reachable-state bound: fixed point of c -> f(c)c + umax
    c = 0.0
    for _ in range(200):
        c = (1.0 - oo1 * sig(w * c + b0)) * c + umax
    cmax = 1.05 * c
    # window: contraction <= f0 per step; 1e-7 truncation target
    L = int(np.ceil(np.log(1e-7) / np.log(min(f0, 0.999))))
    L = int(min(128, max(24, L)))
    cs = np.linspace(0.0, cmax, 4001)
    fs = 1.0 - oo1 * sig(w * cs + b0)
    P = np.polynomial.chebyshev.Chebyshev.fit(cs, fs, 2)
    p0, p1, p2 = (float(v) for v in np.polynomial.chebyshev.cheb2poly(P.convert().coef))
    # Picard seed: f at the mean-input fixed point
    cbar = 0.0
    for _ in range(200):
        cbar = (1.0 - oo1 * sig(w * cbar + b0)) * cbar + umean
    fseed = (p2 * cbar + p1) * cbar + p0
    return L, p0, p1, p2, float(fseed)


def run(inputs, trace=False, L=None, K=4):
    from concourse.bass_utils import run_bass_kernel_spmd

    x, time_lag, oo1, w, b0 = _params(inputs)
    B, T = x.shape
    Lfit, p0, p1, p2, fseed = _fit(oo1, w, b0, float(x.max()), float(x.mean()))
    if L is None:
        L = Lfit

    key = (B, T, time_lag, L, K, p0, p1, p2, fseed)
    if key not in _CACHE:
        _CACHE[key] = _build(B, T, time_lag, L, K, p0, p1, p2, fseed)
    nc = _CACHE[key]

    n_cores = 8
    in_maps = [{"x": x} for _ in range(n_cores)]
    r = run_bass_kernel_spmd(nc, in_maps, core_ids=list(range(n_cores)), trace=trace)
    res = r.results[0]["out"][0 : B - time_lag, 0:4]  # [R, 4] = [C, oo*C, oo, fC]

    outs = []
    for j in (1, 0, 2, 3):  # -> (h, c, oo, f)
        full = np.zeros((B, 1), dtype=np.float32)
        full[time_lag:, 0] = res[:, j]
        outs.append(full)
    return tuple(outs), r.exec_time_ns


def kernel(**inputs):
    outs, _ = run(inputs)
    return outs
